# revision 76
# baseline (speedup 1.0000x reference)
"""Trainium2 Bass kernel for the gammatone-cochlea + LIF-SNN model.

Pipeline per core (32 of the 256 batch rows, pure data parallel):
  1. Gammatone conv [32ch, 64 taps] via tap-split Hankel matmuls on the
     PE in split-fp16 (audio a = ah + al, kernels g = gh + gl, all fp16;
     y = ah*gh + ah*gl + al*gh exactly to ~1e-7 — the al*gl term is
     ~2^-24 and dropped). 4 batch rows per 128-partition group,
     block-diagonal lhsT, 6 accumulating fp16 matmuls (1 cyc/row) per
     512-sample block vs 2 fp32 matmuls (4 cyc/row): 6 vs 8 cycles.
     Each Hankel plane loads as ONE 3D-AP DMA per strip (HWDGE charges
     ~625ns per DMACopy instruction, so few big DMAs).
  2. ReLU on ScalarE (PSUM -> SBUF copy).
  3. Inner-hair-cell framing: DVE strided block-sums (128-sample blocks),
     env[t] = (S[t] + S[t+1]) / 256.
  4. AuditoryNerve: fused tensor_scalar (mult by per-partition scale,
     is_gt threshold) on a 4x partition-replicated env -> 320 spike rows
     (fp16, exact 0/1), bushy currents via 2-term split-fp16 W_bushy
     matmuls, written t-major into the CURB current buffer.
  5. Stacked LIF wavefront (bushy t=s rows 0:50, ic t=s-3 rows 64:114,
     ac t=s-6 rows 114:124; segment bases partition-group aligned).
     Membrane is stored NEGATED (DMn = spk - m2) so each step is only
     two dependent DVE hops:  m2 = -beta*DMn + cur  ->  DMn' = (m2>thr)
     - m2; the threshold op that materializes spikes feeds only the
     weight matmul and outputs (off the critical chain). One combined
     [114,60] matmul per step computes ic+ac currents; Activation
     stages its PSUM into an SBUF ring (mod 4, 3 steps ahead) and Pool
     stages the bushy current, so the fma reads a single staged SBUF
     tile. Spikes are written to parity-split history tiles to avoid a
     write-after-read hazard against the matmul. Outputs DMA straight
     from the history tiles; the host un-negates omem.
Outputs [10, 124*32] (t-major) per core; host reassembles to [B, T, 10].
"""
import numpy as np
import concourse.bass as bass
import concourse.bacc as bacc
import concourse.mybir as mybir
import concourse.tile as tile
from concourse.bass_utils import run_bass_kernel_spmd

dt = mybir.dt
AF = mybir.ActivationFunctionType
OP = mybir.AluOpType

NCORES = 8
B, N, C, K = 256, 16000, 32, 64
BLOC = B // NCORES            # 32 batch rows per core
WINDOW, STRIDE, T = 256, 128, 124
ANS, HID, OUT = 10, 50, 10
BETA, THR, AN_THR = 0.95, 1.0, 0.5
PAD_L, PAD_R = 31, 33         # SAME padding for K=64: 31 left, 32 right (+1 slack)
NPAD = PAD_L + N + PAD_R      # 16064
FREE = T * BLOC               # 3968 (t-major, b-minor)
NGRP = BLOC // 4              # 8 groups of 4 rows
STRIPS = [2048] * 7 + [1664]  # 4-block strips per group

# jnp.linspace(0.5, 1.5, 10, dtype=f32), bitexact
_SCALES = np.array([0x3F000000, 0x3F1C71C7, 0x3F38E38E, 0x3F555555, 0x3F71C71D,
                    0x3F871C72, 0x3F955556, 0x3FA38E39, 0x3FB1C71D, 0x3FC00000],
                   dtype=np.uint32).view(np.float32)

_NC_CACHE = None


def _build_nc():
    nc = bacc.Bacc("TRN2", target_bir_lowering=False, debug=False,
                   num_devices=NCORES)

    apadh = nc.dram_tensor("apadh", [BLOC, NPAD], dt.float16, kind="ExternalInput")
    apadl = nc.dram_tensor("apadl", [BLOC, NPAD], dt.float16, kind="ExternalInput")
    lw = nc.dram_tensor("lw", [4, 128, 128], dt.float16, kind="ExternalInput")
    wb = nc.dram_tensor("wb", [2, 3, 128, HID], dt.float16, kind="ExternalInput")
    wcomb = nc.dram_tensor("wcomb", [64 + HID, HID + OUT], dt.float32,
                           kind="ExternalInput")
    sv = nc.dram_tensor("sv", [128, 3], dt.float32, kind="ExternalInput")
    selr = nc.dram_tensor("selr", [4, 128, 128], dt.float32, kind="ExternalInput")
    ospk = nc.dram_tensor("ospk", [OUT, FREE], dt.float32, kind="ExternalOutput")
    omem = nc.dram_tensor("omem", [OUT, FREE], dt.float32, kind="ExternalOutput")

    with tile.TileContext(nc) as tc:
        with tc.tile_pool(name="cpool", bufs=1) as cp:
            # lw planes: 0=l1h 1=l2h 2=l1l 3=l2l
            # lwt on the Pool DGE queue: HWDGE round-robins queues, so these
            # interleave with (rather than precede) the first Hankel loads
            lwt = [cp.tile([128, 128], dt.float16, name=f"lw{i}")
                   for i in range(4)]
            for i in range(4):
                nc.gpsimd.dma_start(out=lwt[i][:, :], in_=lw[i, :, :])
            # non-conv weights: tiles allocated here, DMAs issued lazily
            # (spread across group-0 strips) so HWDGE serves the first
            # Hankel loads first
            svt = cp.tile([128, 3], dt.float32)
            wbt = [cp.tile([128, HID], dt.float16, name=f"wbt{h}{i}")
                   for h in range(2) for i in range(3)]
            wcombt = cp.tile([64 + HID, HID + OUT], dt.float32)
            selt = [cp.tile([128, 128], dt.float32, name=f"selt{r}")
                    for r in range(4)]

            for r in range(4):
                nc.scalar.dma_start(out=selt[r][:, :], in_=selr[r, :, :])
            nc.gpsimd.dma_start(out=svt[:, :], in_=sv[:, :])
            for h in range(2):
                for i in range(3):
                    nc.gpsimd.dma_start(out=wbt[h * 3 + i][:, :],
                                        in_=wb[h, i, :, :])
            nc.gpsimd.dma_start(out=wcombt[:, :], in_=wcomb[:, :])

            GF = 496             # per-group free = 4*124
            E4 = cp.tile([128, FREE], dt.float32)     # env, 4x partition-replicated
            S_all = cp.tile([128, NGRP * 126], dt.float32)
            env_all = cp.tile([128, NGRP * T], dt.float32)
            # ---- stacked-SNN state: rows 0:50 bushy, 64:114 ic, 114:124 ac
            # (segment bases must be partition-group aligned: 0 and 64)
            LAG = 3
            SNNW = T + 2 * LAG            # 130 wavefront steps
            SROWS = 64 + HID + OUT        # 124
            NBLK2 = (SNNW + 1) // 2       # blocks per parity tile
            CURB = cp.tile([HID, SNNW * BLOC], dt.float32)
            SPK2 = [cp.tile([SROWS, NBLK2 * BLOC], dt.float32,
                            name=f"spk{p}") for p in range(2)]
            DM = cp.tile([SROWS, (SNNW + 1) * BLOC], dt.float32)


            hkp = tc.alloc_tile_pool(name="hkp", bufs=5)
            ybp = tc.alloc_tile_pool(name="ybp", bufs=12)
            anp = tc.alloc_tile_pool(name="anp", bufs=2)
            sp = tc.alloc_tile_pool(name="snn", bufs=1)
            pss = tc.alloc_tile_pool(name="pss", bufs=1, space="PSUM")
            psp = tc.alloc_tile_pool(name="psp", bufs=1, space="PSUM")

            def conv_group(g):
                """Generator: conv + framing for rows 4g..4g+4; yields per strip."""
                for si, sw in enumerate(STRIPS):
                    s0 = 2048 * si
                    hkh = hkp.tile([128, 2112], dt.float16, tag="hkh", name="hkh")
                    hkl = hkp.tile([128, 2112], dt.float16, tag="hkl", name="hkl")
                    # Hankel: hk[r*32+k, j] = apad[4g+r, s0 + j + k]
                    # one 3D-AP DMA per plane: dims (row r, tap k, col j)
                    for dram, hk in ((apadh, hkh), (apadl, hkl)):
                        src = bass.AP(dram, 4 * g * NPAD + s0,
                                      [[NPAD, 4], [1, 32], [1, sw + 32]])
                        nc.sync.dma_start(out=hk[:, 0:sw + 32], in_=src)
                    nb4 = (sw + 511) // 512
                    accs = []
                    # 6 accumulating fp16 matmuls: gh*ah (l1h/l2h on hkh),
                    # gl*ah (l1l/l2l on hkh), gh*al (l1h/l2h on hkl)
                    terms = [(0, 0), (1, 0), (2, 0), (3, 0), (0, 1), (1, 1)]
                    for b4 in range(nb4):
                        w = min(512, sw - 512 * b4)
                        acc = psp.tile([128, 512], dt.float32, tag=f"acc{b4}",
                                       name="acc")
                        accs.append((acc, w))
                        for ti, (wi, hx) in enumerate(terms):
                            hk = hkh if hx == 0 else hkl
                            off = 512 * b4 + 32 * (wi % 2)
                            nc.tensor.matmul(acc[:, 0:w], lwt[wi][:, :],
                                             hk[:, off:off + w],
                                             start=(ti == 0), stop=(ti == 5))
                    for b4 in range(nb4):
                        acc, w = accs[b4]
                        yb = ybp.tile([128, 512], dt.float32, tag="yb", name="yb")
                        nc.scalar.activation(yb[:, 0:w], acc[:, 0:w], AF.Relu)
                        nblk = w // 128
                        i = 4 * si + b4
                        view = bass.AP(yb.tensor, yb.offset,
                                       [list(yb.ap[0]), [128, nblk], [1, 128]])
                        nc.vector.tensor_reduce(
                            S_all[:, g * 126 + 4 * i: g * 126 + 4 * i + nblk],
                            view, axis=mybir.AxisListType.X, op=OP.add)
                    yield

            def epilogue_slice(g, t0, t1):
                # env[t] = (S[t] + S[t+1]) / 256 for t in [t0, t1), then
                # replicate into E4[u*32+c, g*GF + r*T + t] via selectors
                dtt = t1 - t0
                sg = g * 126 + t0
                eg = g * T + t0
                nc.vector.tensor_tensor(env_all[:, eg:eg + dtt],
                                        S_all[:, sg:sg + dtt],
                                        S_all[:, sg + 1:sg + dtt + 1], OP.add)
                nc.vector.tensor_scalar(env_all[:, eg:eg + dtt],
                                        env_all[:, eg:eg + dtt],
                                        1.0 / 256.0, None, OP.mult)
                shf = pss.tile([128, 4 * dtt], dt.float32, tag="misc",
                               bufs=2, name="shf", padded_shape=[128, 64])
                for r in range(4):
                    nc.tensor.matmul(shf[:, r * dtt:(r + 1) * dtt],
                                     selt[r][:, :], env_all[:, eg:eg + dtt],
                                     start=True, stop=True)
                e4v = E4[:, 0:1]
                dst = bass.AP(e4v.tensor, e4v.offset + g * GF + t0,
                              [list(e4v.ap[0]), [T, 4], [1, dtt]])
                src = bass.AP(shf.tensor, shf.offset,
                              [list(shf.ap[0]), [dtt, 4], [1, dtt]])
                nc.scalar.activation(dst, src, AF.Copy)

            def an_slice(g, t0, t1):
                """AN + bushy currents for group g, t in [t0,t1) -> CURB."""
                dtt = t1 - t0
                e4v = E4[:, 0:1]
                e4s = bass.AP(e4v.tensor, e4v.offset + g * GF + t0,
                              [list(e4v.ap[0]), [T, 4], [1, dtt]])
                ps_cb = pss.tile([HID, 4 * dtt], dt.float32, tag="misc",
                                 bufs=2, name="ps_cb", padded_shape=[HID, 64])
                for ch in range(3):
                    an = anp.tile([128, 4 * dtt], dt.float16, tag="an",
                                  name="an", padded_shape=[128, 64])
                    nc.vector.tensor_scalar(an[:, :], e4s,
                                            svt[:, ch:ch + 1], AN_THR,
                                            OP.mult, OP.is_gt)
                    for h in range(2):
                        nc.tensor.matmul(ps_cb[:, :], wbt[h * 3 + ch][:, :],
                                         an[:, :],
                                         start=(ch == 0 and h == 0),
                                         stop=(ch == 2 and h == 1))
                # ps_cb col = r*dtt + tl  ->  CURB col = (t0+tl)*32 + 4g + r
                cur_view = CURB[0:HID, 0:1]
                dst = bass.AP(cur_view.tensor,
                              cur_view.offset + t0 * BLOC + 4 * g,
                              [list(cur_view.ap[0]), [BLOC, dtt], [1, 4]])
                src = bass.AP(ps_cb.tensor, ps_cb.offset,
                              [list(ps_cb.ap[0]), [1, dtt], [dtt, 4]])
                nc.scalar.activation(dst, src, AF.Copy)

            # -------- stacked-LIF wavefront machinery: bushy t=s, ic t=s-3,
            # ac t=s-6. DM holds NEGATED membrane (DMn = spk - m2) so each
            # step is two dependent DVE hops; host negates omem. SPK2 parity
            # split kills the isgt(s+1) <- mm(s) WAR. Act stages mm PSUM
            # into an SBUF ring so the fma never reads PSUM.
            nc.vector.memset(DM[:, 0:BLOC], 0.0)
            nc.vector.memset(CURB[:, T * BLOC:SNNW * BLOC], 0.0)
            ps2 = tc.alloc_tile_pool(name="ps2", bufs=1, space="PSUM")
            stage_ring = {}
            nstg = [0]

            def new_stage(tgt):
                stg = sp.tile([SROWS, BLOC], dt.float32, tag="stg", bufs=4,
                              name="stg")
                if nstg[0] < 4:
                    nc.vector.memset(stg[:, :], 0.0)
                    nstg[0] += 1
                nc.gpsimd.tensor_copy(
                    out=stg[0:HID, :],
                    in_=CURB[:, tgt * BLOC:(tgt + 1) * BLOC])
                stage_ring[tgt] = stg
                return stg

            def snn_step(s):
                c0, c1 = s * BLOC, (s + 1) * BLOC
                sc = (s // 2) * BLOC
                spk = SPK2[s % 2]
                stg = stage_ring.pop(s)
                m2 = sp.tile([SROWS, BLOC], dt.float32, tag="m2", bufs=2,
                             name="m2")
                nc.vector.scalar_tensor_tensor(
                    m2[:, :], DM[:, c0:c1], -BETA, stg[:, :],
                    OP.mult, OP.add)
                nc.vector.scalar_tensor_tensor(
                    DM[:, c1:c1 + BLOC], m2[:, :], THR, m2[:, :],
                    OP.is_gt, OP.subtract)
                nc.vector.tensor_scalar(spk[:, sc:sc + BLOC], m2[:, :], THR,
                                        None, OP.is_gt)
                if s < SNNW - LAG:
                    stg2 = new_stage(s + LAG)
                    pm = ps2.tile([HID + OUT, BLOC], dt.float32, tag="pm",
                                  bufs=2, name="pm")
                    nc.tensor.matmul(pm[:, :], wcombt[:, :],
                                     spk[0:64 + HID, sc:sc + BLOC],
                                     start=True, stop=True)
                    nc.scalar.activation(stg2[64:SROWS, :], pm[:, :],
                                         AF.Copy)

            # -------- strip-major driver: conv strip si for ALL groups, then
            # env/AN for the newly-completed t range, then SNN steps lagging
            # one strip behind -- the LIF wavefront hides under the conv.
            gens = [conv_group(g) for g in range(NGRP)]
            t_done = 0        # env/AN coverage (same for all groups)
            s_done = 0        # SNN steps issued
            prev_hi = 0
            last = len(STRIPS) - 1
            for si in range(len(STRIPS)):
                # SNN steps FIRST: their deps (AN coverage prev_hi) were
                # issued last strip, so they sit ahead of this strip's
                # framing in the in-order DVE queue and execute during it
                for s in range(s_done, prev_hi):
                    snn_step(s)
                s_done = prev_hi
                for g in range(NGRP):
                    next(gens[g])
                    # last strip: slice group g-1 (its framing has had a
                    # full group window to drain, so no PE queue stall)
                    if si == last and g >= 1:
                        epilogue_slice(g - 1, t_done, T)
                        an_slice(g - 1, t_done, T)
                if si == last:
                    epilogue_slice(NGRP - 1, t_done, T)
                    an_slice(NGRP - 1, t_done, T)
                    t_hi = T
                else:
                    t_hi = min(T, 16 * si + 15)
                    for g in range(NGRP):
                        epilogue_slice(g, t_done, t_hi)
                    for g in range(NGRP):
                        an_slice(g, t_done, t_hi)
                if si == 0:
                    for tgt in range(LAG):
                        new_stage(tgt)
                prev_hi = t_hi
                t_done = t_hi
            for s in range(s_done, SNNW):
                snn_step(s)

            # ospk col t*32+b: t even -> SPK2[0] block 3+t/2 (s=t+6);
            # t odd -> SPK2[1] block 3+(t-1)/2. omem from DM directly.
            for par in range(2):
                src = SPK2[par][64 + HID:SROWS,
                               LAG * BLOC:(LAG + T // 2) * BLOC]
                dst = bass.AP(ospk, par * BLOC,
                              [[FREE, OUT], [2 * BLOC, T // 2], [1, BLOC]])
                nc.sync.dma_start(out=dst, in_=src)
            nc.sync.dma_start(
                out=omem[:, :],
                in_=DM[64 + HID:SROWS,
                       (2 * LAG + 1) * BLOC:(2 * LAG + 1 + T) * BLOC])

            ps2.release()
            psp.release()
            pss.release()
            sp.release()
            anp.release()
            ybp.release()
            hkp.release()

    nc.finalize()
    return nc


def _prep_inputs(audio, gt_kernels, W_bushy, W_ic, W_ac):
    audio = np.ascontiguousarray(audio, dtype=np.float32)
    gt = np.ascontiguousarray(gt_kernels, dtype=np.float32)
    Wb = np.ascontiguousarray(W_bushy, dtype=np.float32)

    gth = gt.astype(np.float16)
    gtl = (gt - gth.astype(np.float32)).astype(np.float16)
    lw = np.zeros((4, 128, 128), np.float16)
    for r in range(4):
        # lhsT[r*32+k, r*32+c] = gt[c, k]
        sl = slice(r * 32, r * 32 + 32)
        lw[0, sl, sl] = gth[:, 0:32].T
        lw[1, sl, sl] = gth[:, 32:64].T
        lw[2, sl, sl] = gtl[:, 0:32].T
        lw[3, sl, sl] = gtl[:, 32:64].T

    Wbh = Wb.astype(np.float16)
    Wbl = (Wb - Wbh.astype(np.float32)).astype(np.float16)
    wb = np.zeros((2, 3, 128, HID), np.float16)
    sv = np.zeros((128, 3), np.float32)
    for ch in range(3):
        for u in range(4):
            a = ch * 4 + u
            if a >= ANS:
                continue
            # wb[:, ch, u*32+c, h] = W_bushy[h, c*10 + a]
            wb[0, ch, u * 32:u * 32 + 32, :] = Wbh[:, a::ANS].T
            wb[1, ch, u * 32:u * 32 + 32, :] = Wbl[:, a::ANS].T
            sv[u * 32:u * 32 + 32, ch] = _SCALES[a]
    selr = np.zeros((4, 128, 128), np.float32)
    for r in range(4):
        for u in range(4):
            for c in range(32):
                selr[r, r * 32 + c, u * 32 + c] = 1.0
    wcomb = np.zeros((64 + HID, HID + OUT), np.float32)
    wcomb[0:HID, 0:HID] = W_ic.T
    wcomb[64:64 + HID, HID:HID + OUT] = W_ac.T

    ah = audio.astype(np.float16)
    al = (audio - ah.astype(np.float32)).astype(np.float16)
    in_maps = []
    for c in range(NCORES):
        rs = slice(c * BLOC, (c + 1) * BLOC)
        apadh = np.zeros((BLOC, NPAD), np.float16)
        apadh[:, PAD_L:PAD_L + N] = ah[rs]
        apadl = np.zeros((BLOC, NPAD), np.float16)
        apadl[:, PAD_L:PAD_L + N] = al[rs]
        in_maps.append({"apadh": apadh, "apadl": apadl, "lw": lw, "wb": wb,
                        "wcomb": wcomb, "sv": sv, "selr": selr})
    return in_maps


def kernel(audio, gt_kernels, W_bushy, W_ic, W_ac, _trace=False):
    global _NC_CACHE
    if _NC_CACHE is None:
        _NC_CACHE = _build_nc()
    nc = _NC_CACHE
    in_maps = _prep_inputs(audio, gt_kernels, W_bushy, W_ic, W_ac)
    res = run_bass_kernel_spmd(nc, in_maps, core_ids=list(range(NCORES)),
                               trace=_trace)
    spk = np.empty((B, T, OUT), np.float32)
    mem = np.empty((B, T, OUT), np.float32)
    for c in range(NCORES):
        # [o, t*32+b] -> [b, t, o]
        spk[c * BLOC:(c + 1) * BLOC] = (
            res.results[c]["ospk"].reshape(OUT, T, BLOC).transpose(2, 1, 0))
        # device stores negated membrane (spk - m2); undo the sign here
        mem[c * BLOC:(c + 1) * BLOC] = (
            -res.results[c]["omem"].reshape(OUT, T, BLOC).transpose(2, 1, 0))
    kernel._last_results = res
    return spk, mem

